# revision 14
# baseline (speedup 1.0000x reference)
"""Causal multi-head attention (b=2, n=2048, dim=1024, 16 heads) on 8 trn2
NeuronCores.

Sharding: core j = 4*g + r owns batch g and heads 4r..4r+3 (tensor parallel
over heads within each batch's 4-core group). All matmul-path data is bf16
(PSUM accumulation fp32); biases/normalization fp32.

Per core:
  P1  projects q/k (transposed layout [head_dim, tokens]) and v (natural
      [tokens, head_dim], ones-augmented) for its 4 heads from x^T. nt-outer
      loop + chunked x loads so compute starts as soon as the first token
      block's k-tiles land.
  P2  causal attention per head pair in S^T orientation: d=64 contraction
      row-packed 2 heads/matmul-pair (concurrent PE row-tiles), exp without
      max subtraction, triangular mask on diagonal tiles, O'^T = V_aug.T @
      expS^T in PSUM (row 64 = softmax denominator Z). 1/Z via
      reciprocal_approx_fast straight off PSUM, partition-broadcast via a
      DRAM bounce, normalize straight off PSUM into bf16.
  A2A per 4-core group (replica groups [0..3], [4..7]), one per head pair so
      the first overlaps the second pair's attention; 512KB bf16 payload.
  P3  out = A^T.T @ Wout for this core's 512-token block, plus biases.
Host: transposes x per batch, slices weights per head group, bf16-casts the
matmul-path inputs, gathers the 8 [512, 1024] row blocks into the output.
"""
import numpy as np
import ml_dtypes

import concourse.bass as bass
import concourse.mybir as mybir
import concourse.tile as tile
from concourse.bass import AP, ds
from concourse.bass_utils import run_bass_kernel_spmd
from concourse.vector_clock import ScopedClock

F32 = mybir.dt.float32
BF16 = mybir.dt.bfloat16
EXP = mybir.ActivationFunctionType.Exp

N_CORES = 8
B, N, DIM, H = 2, 2048, 1024, 16
D = DIM // H                 # 64
HL = 4                       # heads per core
KT = DIM // 128              # 8 contraction k-tiles
NJ = N // 128                # 16 key tiles per batch
NI = N // 512                # 4 query i-blocks per batch
SCALE = float(D) ** -0.5
VS = 128                     # per-head stride in v tiles (keeps lhsT aligned)


def _split_multi_waits(nc):
    """This walrus build rejects instructions carrying more than one sync
    wait. Hoist extra waits onto same-engine NoOps inserted directly before
    the offending instruction (engines execute their stream in order, so
    this preserves semantics)."""
    n = 0
    for f in nc.m.functions:
        for bb in f.blocks:
            insts = bb.instructions
            out = []
            changed = False
            for inst in insts:
                si = inst.sync_info
                waits = list(si.on_wait) if si is not None and si.on_wait else []
                if len(waits) > 1:
                    changed = True
                    for w in waits[:-1]:
                        nop = mybir.InstNoOp(name=f"I-waitfix-{n}", ins=[],
                                             outs=[])
                        n += 1
                        nop.engine = inst.engine
                        nop.sync_info = mybir.SyncInfo(on_wait=[w],
                                                       on_update=[])
                        out.append(nop)
                    si.on_wait = waits[-1:]
                out.append(inst)
            if changed:
                insts[:] = out
    return n


class _TC(tile.TileContext):
    """Tail drain in this walrus build only supports one sync-wait per CTRL
    instruction; spread the residual global-clock waits over SP nops, and
    split any remaining multi-wait instructions after scheduling."""

    def _drain_and_barrier(self, tick_clock, wait_clock):
        nop = self.nc.sync.nop()
        wait_clock.add_sem_waits(nop.ins, ScopedClock({None: tick_clock.global_clock}))
        si = nop.ins.sync_info
        waits = list(si.on_wait or []) if si is not None else []
        if len(waits) > 1:
            si.on_wait = waits[:1]
            for w in waits[1:]:
                extra = self.nc.sync.nop()
                extra.ins.sync_info = mybir.SyncInfo(on_wait=[w], on_update=[])
        self.nc.sync.drain()
        self.nc.all_engine_barrier()
        assert self.sems is not None
        popped = self.nc._tile_sem_poison_stack.pop()
        assert popped is self._sem_poison
        self.nc.clear_and_free_semaphores(list(self.sems.allocated().values()))
        self.nc.all_engine_barrier()

    def __exit__(self, exc_type, exc_val, exc_tb):
        r = super().__exit__(exc_type, exc_val, exc_tb)
        if exc_type is None:
            _split_multi_waits(self.nc)
        return r


def _bcast(src_dram_row, parts):
    """DRAM [1, n] row -> AP replicating it over `parts` partitions (step-0
    leading dim; only legal for DRAM sources)."""
    return AP(src_dram_row.tensor, src_dram_row.offset,
              [[0, parts]] + list(src_dram_row.ap)[1:])


def _build():
    nc = bass.Bass(trn_type="TRN2", target_bir_lowering=False, debug=False,
                   num_devices=N_CORES)
    dt = F32
    # pre-tiled on host: [128, KT*width] rows are fully linear so the bulk
    # DMAs run at line rate instead of 1KB-descriptor rate
    xt_d = nc.dram_tensor("xt", [128, KT * N], BF16, kind="ExternalInput").ap()
    wq_d = nc.dram_tensor("wq", [128, KT * HL * D], BF16, kind="ExternalInput").ap()
    wk_d = nc.dram_tensor("wk", [128, KT * HL * D], BF16, kind="ExternalInput").ap()
    wv_d = nc.dram_tensor("wv", [128, KT * HL * D], BF16, kind="ExternalInput").ap()
    wout_d = nc.dram_tensor("wout", [128, KT * DIM], BF16, kind="ExternalInput").ap()
    bq_d = nc.dram_tensor("bq", [HL * D, 1], dt, kind="ExternalInput").ap()
    bk_d = nc.dram_tensor("bk", [HL * D, 1], dt, kind="ExternalInput").ap()
    bv_d = nc.dram_tensor("bv", [1, HL * D], dt, kind="ExternalInput").ap()
    bout_d = nc.dram_tensor("bout", [1, DIM], dt, kind="ExternalInput").ap()
    mask_d = nc.dram_tensor("mask", [128, 128], BF16, kind="ExternalInput").ap()
    ones_d = nc.dram_tensor("ones", [1, HL], BF16, kind="ExternalInput").ap()
    out_d = nc.dram_tensor("out", [N // HL, DIM], dt, kind="ExternalOutput").ap()

    with _TC(nc) as tc, \
            nc.allow_low_precision(reason="bf16 matmul data path"):
        _body(nc, tc, xt_d, wq_d, wk_d, wv_d, wout_d, bq_d, bk_d, bv_d,
              bout_d, mask_d, ones_d, out_d)
    return nc


def _body(nc, tc, xt_d, wq_d, wk_d, wv_d, wout_d, bq_d, bk_d, bv_d, bout_d,
          mask_d, ones_d, out_d):
    mm = nc.tensor.matmul
    with tc.tile_pool(name="persist", bufs=1) as pers:
        # Persistent SBUF: q^T/k^T per head pair, v (ones-augmented) per
        # 128-token tile, mask, biases.
        qt = [pers.tile([128, N], BF16, tag=f"qt{p}", name=f"qt{p}") for p in (0, 1)]
        kt = [pers.tile([128, N], BF16, tag=f"kt{p}", name=f"kt{p}") for p in (0, 1)]
        vt = [pers.tile([128, HL * VS], BF16, tag=f"v{t}", name=f"v{t}")
              for t in range(NJ)]
        mask_sb = pers.tile([128, 128], BF16, tag="mask", name="mask_sb")
        bqc = pers.tile([128, 2], F32, tag="bqc", name="bqc")
        bkc = pers.tile([128, 2], F32, tag="bkc", name="bkc")
        bvb = pers.tile([128, HL * D], F32, tag="bvb", name="bvb")
        boutb = pers.tile([128, DIM], F32, tag="boutb", name="boutb")
        ones_sb = pers.tile([128, HL], BF16, tag="ones", name="ones_sb")

        pid = nc.sync.partition_id()
        gsel = nc.sync.snap(pid // 4, min_val=0, max_val=1)

        with tc.tile_pool(name="p3w", bufs=1) as p3w:
            wout_sb = p3w.tile([128, KT, DIM], BF16, tag="wout", name="wout_sb")

            # ---- P1 (projections) interleaved with P2 pp=0 attention ----
            # Attention i-block I only needs q/k/v token blocks <= I, so the
            # blocks alternate: P1(0), P1(1), A(0,0), P1(2), A(0,1), ... and
            # the tile scheduler fills exp-bound attention stretches with
            # projection matmuls. QK scores and the projection psums share
            # one PSUM tag ("s") to stay within the 8 banks.
            with (tc.tile_pool(name="p1s", bufs=1) as p1s,
                  tc.tile_pool(name="p2s", bufs=4) as p2s,
                  tc.tile_pool(name="p2n", bufs=3) as p2n,
                  tc.tile_pool(name="dram", bufs=1, space="DRAM") as dram,
                  tc.tile_pool(name="p2d", bufs=2, space="DRAM") as p2d,
                  tc.tile_pool(name="sp", bufs=2, space="PSUM") as sp,
                  tc.tile_pool(name="op", bufs=1, space="PSUM") as op):
                w_sb = {}
                for nm in ("wq", "wk", "wv"):
                    w_sb[nm] = p1s.tile([128, KT, HL * D], BF16, tag=nm,
                                        name=f"{nm}_sb")
                xt_sb = p1s.tile([128, KT, N], BF16, tag="xt", name="xt_sb")
                xt_dv = xt_d.rearrange("p (k n) -> p k n", k=KT)
                # load order: the first projection psum needs all of wq + the
                # first token-block columns of every xt k-tile. Split x by
                # (k-tile, half) and alternate the two HWDGE queues;
                # constants ride the serial SWDGE stream on gpsimd.
                nc.sync.dma_start(w_sb["wq"][:],
                                  wq_d.rearrange("p (k e) -> p k e", k=KT))
                nc.scalar.dma_start(w_sb["wk"][:],
                                    wk_d.rearrange("p (k e) -> p k e", k=KT))
                nc.gpsimd.dma_start(w_sb["wv"][:],
                                    wv_d.rearrange("p (k e) -> p k e", k=KT))
                for hf in (0, 1):
                    cols = slice(1024 * hf, 1024 * (hf + 1))
                    for kk in range(KT):
                        eng = nc.sync if (kk % 2 == 0) else nc.scalar
                        eng.dma_start(xt_sb[:, kk, cols], xt_dv[:, kk, cols])
                nc.gpsimd.dma_start(bqc[:],
                                    bq_d.rearrange("(m p) o -> p (m o)", p=128))
                nc.gpsimd.dma_start(bkc[:],
                                    bk_d.rearrange("(m p) o -> p (m o)", p=128))
                nc.gpsimd.dma_start(mask_sb[:], mask_d[:])
                nc.gpsimd.dma_start(bvb[:], _bcast(bv_d[0:1, :], 128))
                nc.gpsimd.dma_start(ones_sb[:], _bcast(ones_d[0:1, :], 128))
                nc.gpsimd.dma_start(boutb[:], _bcast(bout_d[0:1, :], 128))

                # one 8-core AllToAll per head pair so the first overlaps the
                # second pair's attention. Chunk 4g+r (128 rows = 2 heads x
                # 64 dims) lands on core (g, r); each core writes only its
                # own group's chunk positions (gsel-derived dynamic offset),
                # the other group's chunks carry don't-care bytes.
                a2a_in = [dram.tile([1024, 512], BF16, name=f"a2a_in{h}")
                          for h in (0, 1)]
                a2a_out = [dram.tile([1024, 512], BF16, name=f"a2a_out{h}")
                           for h in (0, 1)]

                def p1_block(nt):
                    for w, bcol, dst, mt in (("wq", bqc, qt, 0),
                                             ("wq", bqc, qt, 1),
                                             ("wk", bkc, kt, 0),
                                             ("wk", bkc, kt, 1)):
                        ps = sp.tile([128, 1024], F32, tag="s", name="ps_qk")
                        for kk in range(KT):
                            mm(ps[:, 0:512],
                               w_sb[w][:, kk, 128 * mt:128 * mt + 128],
                               xt_sb[:, kk, 512 * nt:512 * nt + 512],
                               start=(kk == 0), stop=(kk == KT - 1))
                        nc.vector.tensor_scalar_add(
                            dst[mt][:, 512 * nt:512 * nt + 512], ps[:, 0:512],
                            bcol[:, mt:mt + 1])
                    for tt in range(4 * nt, 4 * nt + 4):
                        ps = sp.tile([128, HL * D], F32, tag="pv", name="ps_v")
                        for kk in range(KT):
                            mm(ps[:],
                               xt_sb[:, kk, 128 * tt:128 * tt + 128],
                               w_sb["wv"][:, kk, :],
                               start=(kk == 0), stop=(kk == KT - 1))
                        vv = vt[tt].rearrange("p (h x) -> p h x", x=VS)
                        nc.vector.tensor_add(
                            vv[:, :, 0:D],
                            ps.rearrange("p (h x) -> p h x", x=D),
                            bvb.rearrange("p (h x) -> p h x", x=D))
                        nc.vector.tensor_copy(
                            vv[:, :, D:D + 1],
                            ones_sb.rearrange("p (h o) -> p h o", o=1))

                def attn_block(pp, I):
                    i0 = 512 * I
                    last = 4 * I + 3
                    po = op.tile([D + 1, 1024], F32, tag="po", name="po")
                    for jj in range(4 * I + 4):
                        di = jj - 4 * I
                        f0 = 128 * di if di >= 0 else 0
                        ps = sp.tile([128, 1024], F32, tag="s", name="ps_s")
                        mm(ps[:, f0:512],
                           kt[pp][0:64, 128 * jj:128 * jj + 128],
                           qt[pp][0:64, i0 + f0:i0 + 512],
                           start=True, stop=True)
                        mm(ps[:, 512 + f0:1024],
                           kt[pp][64:128, 128 * jj:128 * jj + 128],
                           qt[pp][64:128, i0 + f0:i0 + 512],
                           start=True, stop=True)
                        e = p2s.tile([128, 1024], BF16, tag="e", name="e_s")
                        ev = e.rearrange("p (h x) -> p h x", x=512)
                        pv2 = ps.rearrange("p (h x) -> p h x", x=512)
                        nc.scalar.activation(ev[:, :, f0:512],
                                             pv2[:, :, f0:512], EXP,
                                             scale=SCALE)
                        if di >= 0:
                            nc.vector.tensor_mul(
                                ev[:, :, f0:f0 + 128],
                                ev[:, :, f0:f0 + 128],
                                mask_sb[:, None, :].to_broadcast((128, 2, 128)))
                        vv = vt[jj].rearrange("p (h x) -> p h x", x=VS)
                        mm(po[:, f0:512], vv[:, 2 * pp, 0:D + 1],
                           e[:, f0:512],
                           start=(jj == 0), stop=(jj == last))
                        mm(po[:, 512 + f0:1024], vv[:, 2 * pp + 1, 0:D + 1],
                           e[:, 512 + f0:1024],
                           start=(jj == 0), stop=(jj == last))
                    # normalization: Z sits in row 64 of the O' psum. DVE
                    # reciprocal on one partition is ~6.4ns/elem, so bounce Z
                    # through DRAM to spread the 1024 values over 128
                    # partitions, reciprocal there (~0.2us), bounce back
                    # partition-broadcast (step-0 DMA), then scale O' into
                    # bf16 and stream this chunk to the a2a buffer.
                    oc = p2n.tile([65, 1024], F32, tag="oc", name="oc")
                    nc.vector.tensor_copy(oc[:], po[:])
                    zdram = p2d.tile([1, 1024], F32, tag="zdram", name="zdram")
                    nc.gpsimd.dma_start(zdram[0:1, :], oc[64:65, :])
                    z128 = p2n.tile([128, 8], F32, tag="z128", name="z128")
                    nc.gpsimd.dma_start(
                        z128[:], zdram.rearrange("o (p e) -> (o p) e", p=128))
                    zr = p2n.tile([128, 8], F32, tag="zr", name="zr")
                    nc.vector.reciprocal(zr[:], z128[:])
                    zrd = p2d.tile([1, 1024], F32, tag="zrd", name="zrd")
                    nc.sync.dma_start(
                        zrd.rearrange("o (p e) -> (o p) e", p=128), zr[:])
                    rzb = p2n.tile([64, 1024], F32, tag="rzb", name="rzb")
                    nc.sync.dma_start(rzb[:], _bcast(zrd[0:1, :], 64))
                    st = p2n.tile([64, 1024], BF16, tag="st", name="st")
                    nc.vector.tensor_mul(st[:], oc[0:64, :], rzb[:])
                    a2v = a2a_in[pp].rearrange("(q h p) c -> p h q c",
                                               h=2, p=64)
                    nc.sync.dma_start(
                        a2v[:, :, ds(gsel * 4 + I, 1), :],
                        st.rearrange("p (h c) -> p h c", h=2))

                p1_block(0)
                p1_block(1)
                attn_block(0, 0)
                p1_block(2)
                attn_block(0, 1)
                p1_block(3)
                attn_block(0, 2)
                attn_block(0, 3)
                nc.gpsimd.collective_compute(
                    "AllToAll", mybir.AluOpType.bypass,
                    replica_groups=[list(range(N_CORES))],
                    ins=[a2a_in[0].opt()], outs=[a2a_out[0].opt()])
                nc.scalar.dma_start(wout_sb[:],
                                    wout_d.rearrange("p (k c) -> p k c", k=KT))
                for I in range(NI):
                    attn_block(1, I)
                nc.gpsimd.collective_compute(
                    "AllToAll", mybir.AluOpType.bypass,
                    replica_groups=[list(range(N_CORES))],
                    ins=[a2a_in[1].opt()], outs=[a2a_out[1].opt()])

            # ---------------- P3: output projection ----------------
            with (tc.tile_pool(name="p3s", bufs=2) as p3s,
                  tc.tile_pool(name="p3p", bufs=8, space="PSUM") as p3p):
                pouts = {}
                for h in (0, 1):
                    atf = p3s.tile([128, 4, 512], BF16, tag=f"atf{h}",
                                   name=f"atf{h}", bufs=1)
                    av = a2a_out[h].rearrange("(G k p) c -> p G k c",
                                              k=4, p=128)
                    nc.sync.dma_start(atf[:], av[:, ds(gsel, 1), :, :])
                    for it in range(4):
                        for ct in range(2):
                            if h == 0:
                                pouts[(it, ct)] = p3p.tile(
                                    [128, 512], F32, tag=f"po{it}{ct}",
                                    name=f"po{it}{ct}", bufs=1)
                            pso = pouts[(it, ct)]
                            for k4 in range(4):
                                kk = 4 * h + k4
                                mm(pso[:],
                                   atf[:, k4, 128 * it:128 * it + 128],
                                   wout_sb[:, kk, 512 * ct:512 * ct + 512],
                                   start=(kk == 0), stop=(kk == KT - 1))
                            if h == 1:
                                osb = p3s.tile([128, 512], F32, tag="osb",
                                               name="osb")
                                nc.vector.tensor_add(
                                    osb[:], pso[:],
                                    boutb[:, 512 * ct:512 * ct + 512])
                                nc.sync.dma_start(
                                    out_d[128 * it:128 * it + 128,
                                          512 * ct:512 * ct + 512], osb[:])


_NC_CACHE = {}

# test-only knobs: set TRACE=True before calling kernel() to profile; the
# BassKernelResults of the last run lands in LAST_RESULT.
TRACE = False
LAST_RESULT = None


def _get_nc():
    if "nc" not in _NC_CACHE:
        _NC_CACHE["nc"] = _build()
    return _NC_CACHE["nc"]


def kernel(x, Wq, bq, Wkv, bkv, Wout, bout):
    x = np.asarray(x, np.float32)
    Wq = np.asarray(Wq, np.float32)
    bq = np.asarray(bq, np.float32)
    Wkv = np.asarray(Wkv, np.float32)
    bkv = np.asarray(bkv, np.float32)
    Wout = np.asarray(Wout, np.float32)
    bout = np.asarray(bout, np.float32)
    bf = ml_dtypes.bfloat16

    def ktile(a):  # [128*KT_rows, width] -> [128, KT_rows*width], row-linear
        kk = a.shape[0] // 128
        return np.ascontiguousarray(
            a.reshape(kk, 128, a.shape[1]).transpose(1, 0, 2).reshape(128, -1)
        ).astype(bf)

    mask = np.triu(np.ones((128, 128), np.float32)).astype(bf)  # c >= p
    xts = [ktile(np.ascontiguousarray(x[g].T)) for g in range(B)]
    # out-proj contraction row order: a2a chunk kk carries heads
    # (4*kk + 2h, 4*kk + 2h + 1) -> permute Wout rows to match
    wout_perm = np.concatenate(
        [Wout[256 * kk + 128 * h:256 * kk + 128 * h + 128]
         for h in (0, 1) for kk in range(4)])
    wout_t = ktile(wout_perm)
    in_maps = []
    for j in range(N_CORES):
        g, r = divmod(j, 4)
        cols = slice(HL * D * r, HL * D * (r + 1))
        in_maps.append({
            "xt": xts[g],
            "wq": ktile(Wq[:, cols]),
            "wk": ktile(Wkv[:, 0:DIM][:, cols]),
            "wv": ktile(Wkv[:, DIM:2 * DIM][:, cols]),
            "wout": wout_t,
            "bq": np.ascontiguousarray(bq[cols][:, None]),
            "bk": np.ascontiguousarray(bkv[0:DIM][cols][:, None]),
            "bv": np.ascontiguousarray(bkv[DIM:2 * DIM][cols][None, :]),
            "bout": np.ascontiguousarray(bout[None, :]),
            "mask": mask,
            "ones": np.ones((1, HL), bf),
        })
    res = run_bass_kernel_spmd(_get_nc(), in_maps, list(range(N_CORES)),
                               trace=TRACE)
    global LAST_RESULT
    LAST_RESULT = res
    out = np.empty((B, N, DIM), np.float32)
    for j in range(N_CORES):
        g, r = divmod(j, 4)
        out[g, 512 * r:512 * (r + 1)] = res.results[j]["out"]
    return out
